# revision 52
# baseline (speedup 1.0000x reference)
"""Trainium2 Bass kernel for nn_CausalMemory (reverse-causal decayed attention).

Math: out = ((qh @ xb.T) * W) @ xb @ VOB, where xb = x @ basis (rank-128),
qh = xb @ (Qc.T Kc), VOB = (Vc.T Oc) basis.T * out_scale, and
W[t,s] = decay^(s-t-1) for s>t else 0 (strictly-future attention).
decay^128 ~ 2e-3, so attention is windowed to the next chunk of 128.

Sharding: 8 cores = batch(4) x sequence-halves(2). Each core handles 2048
query tokens; its key range extends 128 tokens past the query range
(zero-padded at the end of the sequence = exact truncation).

v2: warm-up matmuls beat the HAM cold clock; one merged const DMA; flat
input block DMAs; paired score chunks with one fused mask-multiply each;
evacuation split across scalar+vector; partition-major output layout so the
output DMA is fully contiguous, issued via sync HWDGE.
"""

import numpy as np
import ml_dtypes

B, T, C = 4, 4096, 512
TQ = 2048           # query tokens per core
CH = 128            # chunk
TK = TQ + CH        # key tokens per core (one chunk lookahead)
NCH = TK // CH      # key chunks per core (17)
NT = TQ // CH       # query chunks per core (16)

# DMA blocks (token widths); compute sub-blocks are <=512 within each
DBW = [128, 512, 512, 512, 512]
assert sum(DBW) == TK
DBO = [sum(DBW[:b]) for b in range(len(DBW))]

# consts: cb1 (basis | a | ident), cb2 (vob | wmask), both bf16
CB1_W = 512 + 128 + 128
CB2_W = 512 + 512
# single DRAM input: [cb1 | xt_block0 | cb2 | xt_block1 | xt_block2..4]
ALL_W = CB1_W + 4 * DBW[0] + CB2_W + sum(4 * w for w in DBW[1:])
A_XT0 = CB1_W
A_CB2 = CB1_W + 4 * DBW[0]
A_XT = [A_XT0, A_CB2 + CB2_W]
for _w in DBW[1:-1]:
    A_XT.append(A_XT[-1] + 4 * _w)

_CACHE = {}

CFG = {
    "warm": 8,           # warm-up matmuls (N=512 each)
    "out_sc": 7,         # out evac chunks on scalar (of 16); rest vector
    "qh_eng": "v",       # qh evac engine
    "rv_eng": "s",       # rv evac engine
    "xb_eng": "s",
}


def _build():
    import concourse.tile as tile
    from concourse import bacc, mybir

    bf16 = mybir.dt.bfloat16
    f32 = mybir.dt.float32

    nc = bacc.Bacc("TRN2", target_bir_lowering=False, debug=False, num_devices=8)

    xt_ext = nc.declare_dram_parameter("xt", [128, 4 * TK], bf16, isOutput=False)
    f1_ext = nc.declare_dram_parameter(
        "f1", [128, CB1_W + 4 * DBW[0]], bf16, isOutput=False)
    cb2_ext = nc.declare_dram_parameter("cb2", [128, CB2_W], bf16, isOutput=False)
    out_ext = nc.declare_dram_parameter("out", [128, NT * 512], bf16, isOutput=True)

    def _copy(eng, dst, srcap):
        if eng == "v":
            nc.vector.tensor_copy(dst, srcap)
        else:
            nc.scalar.copy(dst, srcap)

    with tile.TileContext(nc) as tc:
        with (
            tc.tile_pool(name="consts", bufs=1) as cpool,
            tc.tile_pool(name="xt", bufs=3) as xtp,
            tc.tile_pool(name="big", bufs=1) as bigp,
            tc.tile_pool(name="st", bufs=5) as stp,
            tc.tile_pool(name="rv", bufs=3) as rvp,
            tc.tile_pool(name="outb", bufs=2) as outp,
            tc.tile_pool(name="ps_xb", bufs=2, space="PSUM") as ps_xb,
            tc.tile_pool(name="ps_misc", bufs=2, space="PSUM") as ps_misc,
            tc.tile_pool(name="ps_st", bufs=2, space="PSUM") as ps_st,
            tc.tile_pool(name="ps_out", bufs=2, space="PSUM") as ps_out,
        ):
            # fused tile: cb1 consts + xt block 0 — one DMA, one receipt
            f1 = cpool.tile([128, CB1_W + 4 * DBW[0]], bf16)
            nc.sync.dma_start(f1[:], f1_ext[:])
            basis_s = f1[:, 0:512]
            a_s = f1[:, 512:640]
            id_s = f1[:, 640:768]
            cb2 = cpool.tile([128, CB2_W], bf16)
            vob_s = cb2[:, 0:512]
            wm_s = cb2[:, 512:1024]   # [X | D | X | D] pattern, 4x128

            # warm-up: keep PE busy from t0 so HAM unthrottles before real work
            if CFG["warm"]:
                warm = cpool.tile([128, 128], bf16)
                warm5 = cpool.tile([128, 512], bf16)
                nc.vector.memset(warm[:], 0)
                nc.vector.memset(warm5[:], 0)
                for wi in range(CFG["warm"]):
                    pw = ps_out.tile([128, 512], f32, tag="pout")
                    nc.tensor.matmul(pw[:], warm[:], warm5[:],
                                     start=True, stop=True)

            # input block DMAs (flat, contiguous); block 0 rode along in f1
            xts = [(f1, CB1_W)]
            for kb in range(1, len(DBW)):
                w = DBW[kb]
                off = DBO[kb]
                xt = xtp.tile([128, 4 * w], bf16, tag="xt")
                nc.sync.dma_start(xt[:], xt_ext[:, 4 * off:4 * (off + w)])
                xts.append((xt, 0))
                if kb == 1:
                    nc.sync.dma_start(cb2[:], cb2_ext[:])

            xb_big = bigp.tile([128, TK], bf16, tag="xb")
            xtok_big = bigp.tile([128, TK], bf16, tag="xtok")
            qh_big = bigp.tile([128, TQ], bf16, tag="qh")
            st_s = {}

            def sub_blocks():
                for kb, bw in enumerate(DBW):
                    o = DBO[kb]
                    while bw > 0:
                        w = min(512, bw)
                        yield kb, o, w
                        o += w
                        bw -= w

            def block_stage(kb, off, w):
                xt, xbase = xts[kb]
                bw = DBW[kb]
                lo = off - DBO[kb]
                pxb = ps_xb.tile([128, w], f32, tag="pxb")
                for sl in range(4):
                    o = xbase + sl * bw + lo
                    nc.tensor.matmul(
                        pxb[:], basis_s[:, sl * 128:(sl + 1) * 128],
                        xt[:, o:o + w],
                        start=(sl == 0), stop=(sl == 3))
                xb = xb_big[:, off:off + w]
                _copy(CFG["xb_eng"], xb, pxb[:])

                ptk = ps_misc.tile([128, w], bf16, tag="pmisc")
                for ci in range(w // 128):
                    nc.tensor.transpose(
                        ptk[:, ci * 128:(ci + 1) * 128],
                        xb[:, ci * 128:(ci + 1) * 128], id_s)
                nc.vector.tensor_copy(xtok_big[:, off:off + w], ptk[:])

                if off < TQ:
                    qw = min(w, TQ - off)
                    pqh = ps_misc.tile([128, qw], f32, tag="pmisc")
                    nc.tensor.matmul(pqh[:], a_s, xb[:, 0:qw],
                                     start=True, stop=True)
                    _copy(CFG["qh_eng"], qh_big[:, off:off + qw], pqh[:])

            def scores_pair(cg):
                # key chunks c0=2cg, c0+1; per chunk 256 cols: [X_c | D_c]
                # X_c multiplies queries of chunk c-1, D_c queries of chunk c
                c0 = 2 * cg
                c1 = min(NCH, c0 + 2)
                pst = ps_st.tile([128, 512], f32, tag="pst")
                st = stp.tile([128, 512], bf16, tag="st")
                for c in range(c0, c1):
                    base = (c - c0) * 256
                    if c == 0:
                        nc.tensor.matmul(pst[:, 128:256],
                                         xb_big[:, 0:128], qh_big[:, 0:128],
                                         start=True, stop=True)
                    elif c == NCH - 1:
                        nc.tensor.matmul(pst[:, base:base + 128],
                                         xb_big[:, c * 128:(c + 1) * 128],
                                         qh_big[:, (c - 1) * 128:c * 128],
                                         start=True, stop=True)
                    else:
                        nc.tensor.matmul(pst[:, base:base + 256],
                                         xb_big[:, c * 128:(c + 1) * 128],
                                         qh_big[:, (c - 1) * 128:(c + 1) * 128],
                                         start=True, stop=True)
                    st_s[c] = st
                lo = 128 if c0 == 0 else 0
                hi = 128 if c1 == NCH and c1 - c0 == 1 else (c1 - c0) * 256
                nc.vector.tensor_mul(st[:, lo:hi], pst[:, lo:hi],
                                     wm_s[:, lo:hi])

            def out_group(g):
                # query chunks 4g..4g+3
                prv = ps_misc.tile([128, 4, 128], f32, tag="pmisc")
                for p in range(4):
                    i = g * 4 + p
                    nc.tensor.matmul(
                        prv[:, p, :],
                        xtok_big[:, i * 128:(i + 1) * 128],
                        st_s[i][:, (i % 2) * 256 + 128:(i % 2) * 256 + 256],
                        start=True, stop=False)
                    j = i + 1
                    nc.tensor.matmul(
                        prv[:, p, :],
                        xtok_big[:, j * 128:(j + 1) * 128],
                        st_s[j][:, (j % 2) * 256:(j % 2) * 256 + 128],
                        start=False, stop=True)
                rvg = rvp.tile([128, 4, 128], bf16, tag="rv")
                _copy(CFG["rv_eng"], rvg[:], prv[:])
                ob = outp.tile([128, 2048], bf16, tag="outb")
                last = g == NT // 4 - 1
                for q in range(4):
                    i = g * 4 + q
                    pout = ps_out.tile([128, 512], f32, tag="pout")
                    nc.tensor.matmul(pout[:], rvg[:, q, :], vob_s,
                                     start=True, stop=True)
                    if last:
                        eng = "s" if q % 2 == 0 else "v"
                    else:
                        eng = "s" if (i * CFG["out_sc"]) % 16 < CFG["out_sc"] \
                            else "v"
                    _copy(eng, ob[:, q * 512:(q + 1) * 512], pout[:])
                    if last:
                        nc.sync.dma_start(
                            out_ext[:, i * 512:(i + 1) * 512],
                            ob[:, q * 512:(q + 1) * 512])
                if not last:
                    nc.sync.dma_start(
                        out_ext[:, g * 2048:(g + 1) * 2048], ob[:])

            emitted_cg = 0
            emitted_g = 0
            n_cg = (NCH + 1) // 2  # 9
            for kb, off, w in sub_blocks():
                block_stage(kb, off, w)
                chunks_done = (off + w) // 128
                while emitted_cg < n_cg and \
                        min(NCH, (emitted_cg + 1) * 2) <= chunks_done:
                    scores_pair(emitted_cg)
                    emitted_cg += 1
                while emitted_g < NT // 4 and \
                        4 * emitted_g + 5 <= emitted_cg * 2:
                    out_group(emitted_g)
                    emitted_g += 1
            while emitted_cg < n_cg:
                scores_pair(emitted_cg)
                emitted_cg += 1
            while emitted_g < NT // 4:
                out_group(emitted_g)
                emitted_g += 1

    nc.compile()
    return nc


def _host_consts(basis, qc, kc, vc, oc, decay_logit, out_scale):
    bf = ml_dtypes.bfloat16
    d = 1.0 / (1.0 + np.exp(-np.float64(decay_logit)))
    basis64 = np.asarray(basis, np.float64)
    A = np.asarray(qc, np.float64).T @ np.asarray(kc, np.float64)
    VOB = (np.asarray(vc, np.float64).T @ np.asarray(oc, np.float64)) \
        @ basis64.T * np.float64(out_scale)
    # wm [128, 512] = [X | D | X | D]
    s_idx = np.arange(CH)[:, None]
    t_idx = np.arange(CH)[None, :]
    X = d ** (CH + s_idx - t_idx - 1)
    D = np.where(s_idx > t_idx, d ** np.maximum(s_idx - t_idx - 1, 0), 0.0)
    wm = np.concatenate([X, D, X, D], axis=1)

    cb1 = np.zeros((128, CB1_W), dtype=bf)
    cb1[:, 0:512] = basis64.astype(np.float32).reshape(4, 128, 128) \
        .transpose(1, 0, 2).reshape(128, 512).astype(bf)
    cb1[:, 512:640] = A.astype(np.float32).astype(bf)
    cb1[:, 640:768] = np.eye(128, dtype=np.float32).astype(bf)
    cb2 = np.zeros((128, CB2_W), dtype=bf)
    cb2[:, 0:512] = VOB.astype(np.float32).astype(bf)
    cb2[:, 512:1024] = wm.astype(np.float32).astype(bf)
    return cb1, cb2


def make_in_maps(x, basis, q_coeffs, k_coeffs, v_coeffs, o_coeffs,
                 decay_logit, out_scale):
    bf = ml_dtypes.bfloat16
    cb1, cb2 = _host_consts(basis, q_coeffs, k_coeffs, v_coeffs, o_coeffs,
                            decay_logit, out_scale)
    x = np.asarray(x, np.float32)
    in_maps = []
    for b in range(B):
        xbT = np.ascontiguousarray(x[b].T)  # [C, T]
        for h in range(2):
            q0 = h * TQ
            xs = np.zeros((C, TK), dtype=np.float32)
            avail = min(TK, T - q0)
            xs[:, :avail] = xbT[:, q0:q0 + avail]
            x4 = xs.reshape(4, 128, TK)
            xt_p = np.zeros((128, 4 * TK), dtype=bf)
            for kb, w in enumerate(DBW):
                off = DBO[kb]
                for s in range(4):
                    xt_p[:, 4 * off + s * w:4 * off + (s + 1) * w] = \
                        x4[s, :, off:off + w]
            f1_p = np.empty((128, CB1_W + 4 * DBW[0]), dtype=bf)
            f1_p[:, 0:CB1_W] = cb1
            w0 = DBW[0]
            for s in range(4):
                f1_p[:, CB1_W + s * w0:CB1_W + (s + 1) * w0] = \
                    x4[s, :, 0:w0]
            in_maps.append({"xt": xt_p, "f1": f1_p, "cb2": cb2})
    return in_maps


def assemble_out(results):
    out = np.zeros((B, T, C), dtype=np.float32)
    for core in range(8):
        b, h = core // 2, core % 2
        r = np.asarray(results[core]["out"]).astype(np.float32)
        out[b, h * TQ:(h + 1) * TQ, :] = \
            r.reshape(128, NT, 512).transpose(1, 0, 2).reshape(TQ, 512)
    return out


def get_nc():
    if "nc" not in _CACHE:
        _CACHE["nc"] = _build()
    return _CACHE["nc"]


def kernel(x, basis, q_coeffs, k_coeffs, v_coeffs, o_coeffs,
           decay_logit, out_scale):
    from concourse.bass_utils import run_bass_kernel_spmd

    nc = get_nc()
    in_maps = make_in_maps(x, basis, q_coeffs, k_coeffs, v_coeffs, o_coeffs,
                           decay_logit, out_scale)
    res = run_bass_kernel_spmd(nc, in_maps, list(range(8)))
    return assemble_out(res.results)
